# revision 3
# baseline (speedup 1.0000x reference)
"""Haar DWT (2x2 block transform) for Trainium2, data-parallel over 8 NeuronCores.

Full input x: (16, 64, 256, 256) fp32 -> output (16, 256, 128, 128) fp32 where
out[b, 4c+k] = subband k of channel c, k in [cA, cH, cV, cD].

Sharding: batch dim 16 -> 2 per core. Per core the (2, 64) batch/channel dims
flatten to exactly 128 images = the SBUF partition dim; each partition owns one
256x256 image laid out contiguously in its free dim.

Per-core pipeline (per 32-row tile of every image):
  1. DMA in  (128, 8192) fp32 -> xt         [nc.sync / HWDGE, 4 MiB contiguous]
  2. VectorE: u = top+bot, v = bot-top      [vertical butterfly, unit stride]
  3. ScalarE: uv *= 0.5                     [folds the Haar 1/2 scale]
  4. VectorE: even+odd -> [cA|cH], odd-even -> [cV|cD], cast to bf16 on write
  5. DMA out (128, 4x2048) bf16 from res to the 4 subband regions in one store

The op is memory-bound; the correctness gate (rel err < 2e-2) leaves room to
store the output as bf16 (rounding error ~2^-9 rel), halving store traffic:
per core 32 MiB in + 16 MiB out against the ~358 GB/s per-NC HBM limit
-> ~141 us floor (vs 187 us with fp32 stores). The host widens bf16 -> fp32
after the gather; DVE does the fp32->bf16 cast for free on the pass-2 write.
Loads ride the SP HWDGE ring and stores the ACT HWDGE ring: rings are
FIFO, so on a single ring the stores would stall the input feed.
"""

import numpy as np

B, C, H, W = 16, 64, 256, 256
N_CORES = 8
B_PER = B // N_CORES  # 2
IMGS = B_PER * C  # 128 images/core = SBUF partitions
IMG_PIX = H * W  # 65536 elements per image
# uniform 32-row tiles measured best on HW (tapered schedules added more
# instructions/semaphores than the shorter ramp/tail repaid)
TILE_ROWS = [32, 32, 32, 32, 32, 32, 32, 32]
assert sum(TILE_ROWS) == H
MAX_K = max(TILE_ROWS) * W  # slot size for the tile pools
SUB = (H // 2) * (W // 2)  # 16384 elements per subband

_CACHE: dict = {}


def build_nc():
    import concourse.bacc as bacc
    import concourse.mybir as mybir
    from concourse.tile import TileContext

    fp32 = mybir.dt.float32
    bf16 = mybir.dt.bfloat16
    # Bacc (not plain Bass): its generate_event_semaphores pass splits
    # multi-sem waits, which the TRN2 static-DMA encoding can't hold.
    nc = bacc.Bacc(target_bir_lowering=False, debug=False)
    x = nc.dram_tensor("x", [IMGS, IMG_PIX], fp32, kind="ExternalInput")
    y = nc.dram_tensor("y", [IMGS, 4 * SUB], bf16, kind="ExternalOutput")
    # y viewed per subband: (128, 4, 16384)
    y_sub = y[:].rearrange("p (k s) -> p k s", k=4)

    with TileContext(nc) as tc:
        with (
            tc.tile_pool(name="xt", bufs=3) as pool_x,
            tc.tile_pool(name="uv", bufs=2) as pool_uv,
            tc.tile_pool(name="res", bufs=2) as pool_res,
        ):
            row0 = 0
            for rows in TILE_ROWS:
                K = rows * W  # free elems / partition this tile
                out_k = K // 4  # out elems / subband / partition this tile
                xt = pool_x.tile([IMGS, MAX_K], fp32)
                nc.sync.dma_start(
                    out=xt[:, 0:K], in_=x[:, row0 * W : row0 * W + K]
                )

                # vertical butterfly: row pairs (2i, 2i+1), unit-stride operands
                xv = xt[:, 0:K].rearrange("p (i w) -> p i w", w=2 * W)
                top = xv[:, :, 0:W]
                bot = xv[:, :, W : 2 * W]
                uv = pool_uv.tile([IMGS, MAX_K], fp32)
                u = uv[:, 0 : K // 2].rearrange("p (i w) -> p i w", w=W)
                v = uv[:, K // 2 : K].rearrange("p (i w) -> p i w", w=W)
                nc.vector.tensor_add(out=u, in0=top, in1=bot)  # a+c, b+d
                nc.vector.tensor_sub(out=v, in0=bot, in1=top)  # c-a, d-b
                # fold the Haar 1/2 on ScalarE, keeping DMAs single-dependency:
                # xt is only ever read by DVE, res only written by DVE.
                nc.scalar.mul(uv[:, 0:K], uv[:, 0:K], 0.5)

                # horizontal butterfly: column pairs; same op serves both halves
                uvp = uv[:, 0:K].rearrange("p (n u) -> p n u", u=2)
                even = uvp[:, :, 0]
                odd = uvp[:, :, 1]
                # pass-2 writes cast fp32 -> bf16 (free on the DVE output path);
                # bf16 res halves the store stream vs fp32
                res = pool_res.tile([IMGS, MAX_K], bf16)
                nc.vector.tensor_add(out=res[:, 0 : K // 2], in0=even, in1=odd)  # [cA|cH]
                nc.vector.tensor_sub(out=res[:, K // 2 : K], in0=odd, in1=even)  # [cV|cD]

                # res = [cA|cH|cV|cD]; one strided store to all 4 subband regions
                o0 = (row0 // 2) * (W // 2)  # out offset within each subband
                dst = y_sub[:, :, o0 : o0 + out_k]  # (128, 4, out_k)
                src = res[:, 0:K].rearrange("p (k o) -> p k o", k=4)
                # stores ride the ACT HWDGE ring so loads stream uninterrupted
                # on the SP HWDGE ring (one ring = FIFO: outs stall the in feed)
                nc.scalar.dma_start(out=dst, in_=src)
                row0 += rows
    # run Bacc's pass pipeline (regalloc, DCE, event-semaphore wait splitting)
    nc.compile()
    return nc


def _get_nc():
    if "nc" not in _CACHE:
        _CACHE["nc"] = build_nc()
    return _CACHE["nc"]


def _unshard(results):
    return np.concatenate(
        [
            np.asarray(r["y"]).astype(np.float32).reshape(B_PER, C * 4, H // 2, W // 2)
            for r in results
        ],
        axis=0,
    )


def kernel(x: np.ndarray) -> np.ndarray:
    from concourse.bass_utils import run_bass_kernel_spmd

    x = np.ascontiguousarray(np.asarray(x), dtype=np.float32)
    assert x.shape == (B, C, H, W), x.shape

    nc = _get_nc()
    in_maps = [
        {"x": x[c * B_PER : (c + 1) * B_PER].reshape(IMGS, IMG_PIX)}
        for c in range(N_CORES)
    ]
    results = run_bass_kernel_spmd(nc, in_maps, core_ids=list(range(N_CORES))).results
    return _unshard(results)


# revision 4
# speedup vs baseline: 1.0845x; 1.0845x over previous
"""Haar DWT (2x2 block transform) for Trainium2, data-parallel over 8 NeuronCores.

Full input x: (16, 64, 256, 256) fp32 -> output (16, 256, 128, 128) fp32 where
out[b, 4c+k] = subband k of channel c, k in [cA, cH, cV, cD].

Sharding: batch dim 16 -> 2 per core. Per core the (2, 64) batch/channel dims
flatten to exactly 128 images = the SBUF partition dim; each partition owns one
256x256 image laid out contiguously in its free dim.

Per-core pipeline (per 32-row tile of every image):
  1. DMA in  (128, 8192) fp32 -> xt            [nc.sync / HWDGE, 4 MiB contiguous]
  2. ScalarE: xb = bf16(0.5 * xt)              [cast + exact pow2 scale in one op]
  3. VectorE: u = top+bot, v = bot-top         [bf16 unit-stride -> 2x packed mode]
  4. VectorE: even+odd -> [cA|cH], odd-even -> [cV|cD]  [bf16, stride-2 -> 1x]
  5. DMA out (128, 4x2048) bf16 from res to the 4 subband regions in one store

The op is memory-bound; the correctness gate (rel err < 2e-2) leaves room to
compute and store in bf16 (total rounding ~3*2^-9 rel), which (a) halves store
traffic: 32 MiB in + 16 MiB out per core vs the ~358 GB/s per-NC HBM limit
-> ~141 us DMA floor, and (b) doubles DVE throughput on the unit-stride pass
via the 2x_1P packed mode, taking DVE (141 us busy in fp32, the previous
pacer) down to ~106 us so the DMA stream is the only critical resource.
The ACT engine (otherwise idle) does the fp32->bf16 cast at ~148 G elem/s;
folding the Haar 1/2 into its activation scale is free and exact.
Loads ride the SP HWDGE ring and stores the ACT HWDGE ring: rings are
FIFO, so on a single ring the stores would stall the input feed.
"""

import numpy as np

B, C, H, W = 16, 64, 256, 256
N_CORES = 8
B_PER = B // N_CORES  # 2
IMGS = B_PER * C  # 128 images/core = SBUF partitions
IMG_PIX = H * W  # 65536 elements per image
TILE_ROWS = [32, 32, 32, 32, 32, 32, 32, 32]
assert sum(TILE_ROWS) == H
MAX_K = max(TILE_ROWS) * W  # slot size for the tile pools
SUB = (H // 2) * (W // 2)  # 16384 elements per subband

_CACHE: dict = {}


def build_nc():
    import concourse.bacc as bacc
    import concourse.mybir as mybir
    from concourse.tile import TileContext

    fp32 = mybir.dt.float32
    bf16 = mybir.dt.bfloat16
    # Bacc (not plain Bass): its generate_event_semaphores pass splits
    # multi-sem waits, which the TRN2 static-DMA encoding can't hold.
    nc = bacc.Bacc(target_bir_lowering=False, debug=False)
    x = nc.dram_tensor("x", [IMGS, IMG_PIX], fp32, kind="ExternalInput")
    y = nc.dram_tensor("y", [IMGS, 4 * SUB], bf16, kind="ExternalOutput")
    # y viewed per subband: (128, 4, 16384)
    y_sub = y[:].rearrange("p (k s) -> p k s", k=4)

    with TileContext(nc) as tc:
        with (
            tc.tile_pool(name="xt", bufs=3) as pool_x,
            tc.tile_pool(name="xb", bufs=2) as pool_xb,
            tc.tile_pool(name="uv", bufs=2) as pool_uv,
            tc.tile_pool(name="res", bufs=2) as pool_res,
        ):
            row0 = 0
            for rows in TILE_ROWS:
                K = rows * W  # free elems / partition this tile
                out_k = K // 4  # out elems / subband / partition this tile
                xt = pool_x.tile([IMGS, MAX_K], fp32)
                nc.sync.dma_start(
                    out=xt[:, 0:K], in_=x[:, row0 * W : row0 * W + K]
                )

                # ACT: cast to bf16 with the Haar 1/2 folded in (exact pow2)
                xb = pool_xb.tile([IMGS, MAX_K], bf16)
                nc.scalar.mul(xb[:, 0:K], xt[:, 0:K], 0.5)

                # vertical butterfly: row pairs (2i, 2i+1); all-bf16 unit-stride
                # operands let the DVE run in 2x packed mode
                xv = xb[:, 0:K].rearrange("p (i w) -> p i w", w=2 * W)
                top = xv[:, :, 0:W]
                bot = xv[:, :, W : 2 * W]
                uv = pool_uv.tile([IMGS, MAX_K], bf16)
                u = uv[:, 0 : K // 2].rearrange("p (i w) -> p i w", w=W)
                v = uv[:, K // 2 : K].rearrange("p (i w) -> p i w", w=W)
                nc.vector.tensor_add(out=u, in0=top, in1=bot)  # a+c, b+d
                nc.vector.tensor_sub(out=v, in0=bot, in1=top)  # c-a, d-b

                # horizontal butterfly: column pairs; same op serves both halves
                uvp = uv[:, 0:K].rearrange("p (n u) -> p n u", u=2)
                even = uvp[:, :, 0]
                odd = uvp[:, :, 1]
                res = pool_res.tile([IMGS, MAX_K], bf16)
                nc.vector.tensor_add(out=res[:, 0 : K // 2], in0=even, in1=odd)  # [cA|cH]
                nc.vector.tensor_sub(out=res[:, K // 2 : K], in0=odd, in1=even)  # [cV|cD]

                # res = [cA|cH|cV|cD]; one strided store to all 4 subband regions
                o0 = (row0 // 2) * (W // 2)  # out offset within each subband
                dst = y_sub[:, :, o0 : o0 + out_k]  # (128, 4, out_k)
                src = res[:, 0:K].rearrange("p (k o) -> p k o", k=4)
                # stores ride the ACT HWDGE ring so loads stream uninterrupted
                # on the SP HWDGE ring (one ring = FIFO: outs stall the in feed)
                nc.scalar.dma_start(out=dst, in_=src)
                row0 += rows
    # run Bacc's pass pipeline (regalloc, DCE, event-semaphore wait splitting)
    nc.compile()
    return nc


def _get_nc():
    if "nc" not in _CACHE:
        _CACHE["nc"] = build_nc()
    return _CACHE["nc"]


def _unshard(results):
    return np.concatenate(
        [
            np.asarray(r["y"]).astype(np.float32).reshape(B_PER, C * 4, H // 2, W // 2)
            for r in results
        ],
        axis=0,
    )


def kernel(x: np.ndarray) -> np.ndarray:
    from concourse.bass_utils import run_bass_kernel_spmd

    x = np.ascontiguousarray(np.asarray(x), dtype=np.float32)
    assert x.shape == (B, C, H, W), x.shape

    nc = _get_nc()
    in_maps = [
        {"x": x[c * B_PER : (c + 1) * B_PER].reshape(IMGS, IMG_PIX)}
        for c in range(N_CORES)
    ]
    results = run_bass_kernel_spmd(nc, in_maps, core_ids=list(range(N_CORES))).results
    return _unshard(results)


# revision 5
# speedup vs baseline: 1.2209x; 1.1258x over previous
"""Haar DWT (2x2 block transform) for Trainium2, data-parallel over 8 NeuronCores.

Full input x: (16, 64, 256, 256) fp32 -> output (16, 256, 128, 128) fp32 where
out[b, 4c+k] = subband k of channel c, k in [cA, cH, cV, cD].

Sharding: batch dim 16 -> 2 per core. Per core the (2, 64) batch/channel dims
flatten to exactly 128 images = the SBUF partition dim; each partition owns one
256x256 image laid out contiguously in its free dim.

Per-core pipeline (per row-tile of every image):
  1. DMA in  (128, K) fp32 -> xt               [nc.sync / HWDGE, contiguous]
  2. ScalarE: xb = bf16(0.5 * xt), de-interleaving even/odd columns
     (strided read, unit-stride write) -> xb = [evens | odds]
  3. VectorE: u_e=te+be, u_o=to+bo, v_e=be-te, v_o=bo-to   [vertical butterfly]
  4. VectorE: cA=u_e+u_o, cH=v_e+v_o, cV=u_o-u_e, cD=v_o-v_e [horizontal]
  5. DMA out (128, 4 x K/4) bf16 from res to the 4 subband regions in one store

The op is memory-bound; the correctness gate (rel err < 2e-2) leaves room to
compute and store in bf16 (total rounding ~3*2^-9 rel), which (a) halves store
traffic: 32 MiB in + 16 MiB out per core vs the ~358 GB/s per-NC HBM limit
-> ~141 us DMA floor, and (b) doubles DVE throughput via the 2x_1P packed
mode -- but only for all-2-byte, all-unit-stride operands. The column
de-interleave done for free inside the ACT cast makes BOTH butterfly passes
unit-stride, so all 8 DVE ops/tile run packed: DVE ~71 us busy, under the
~106 us load stream, leaving DMA as the sole pacer. Folding the Haar 1/2
into the ACT activation scale is free and exact (pow2). First/last tiles are
16 rows to start compute sooner and shorten the drain tail. Loads ride the
SP HWDGE ring and stores the ACT HWDGE ring (one ring = FIFO: stores would
stall the input feed).
"""

import numpy as np

B, C, H, W = 16, 64, 256, 256
N_CORES = 8
B_PER = B // N_CORES  # 2
IMGS = B_PER * C  # 128 images/core = SBUF partitions
IMG_PIX = H * W  # 65536 elements per image
TILE_ROWS = [16, 32, 32, 32, 32, 32, 32, 32, 16]
assert sum(TILE_ROWS) == H
MAX_K = max(TILE_ROWS) * W  # slot size for the tile pools
SUB = (H // 2) * (W // 2)  # 16384 elements per subband

_CACHE: dict = {}


def build_nc():
    import concourse.bacc as bacc
    import concourse.mybir as mybir
    from concourse.tile import TileContext

    fp32 = mybir.dt.float32
    bf16 = mybir.dt.bfloat16
    # Bacc (not plain Bass): its generate_event_semaphores pass splits
    # multi-sem waits, which the TRN2 static-DMA encoding can't hold.
    nc = bacc.Bacc(target_bir_lowering=False, debug=False)
    x = nc.dram_tensor("x", [IMGS, IMG_PIX], fp32, kind="ExternalInput")
    y = nc.dram_tensor("y", [IMGS, 4 * SUB], bf16, kind="ExternalOutput")
    # y viewed per subband: (128, 4, 16384)
    y_sub = y[:].rearrange("p (k s) -> p k s", k=4)

    with TileContext(nc) as tc:
        with (
            tc.tile_pool(name="xt", bufs=3) as pool_x,
            tc.tile_pool(name="xb", bufs=2) as pool_xb,
            tc.tile_pool(name="uv", bufs=2) as pool_uv,
            tc.tile_pool(name="res", bufs=2) as pool_res,
        ):
            row0 = 0
            for rows in TILE_ROWS:
                K = rows * W  # free elems / partition this tile
                q = K // 4  # elems per quarter (= per subband) this tile
                hw = W // 2  # 128: row length after the even/odd split
                xt = pool_x.tile([IMGS, MAX_K], fp32)
                nc.sync.dma_start(
                    out=xt[:, 0:K], in_=x[:, row0 * W : row0 * W + K]
                )

                # ACT: cast to bf16 with the Haar 1/2 folded in (exact pow2),
                # de-interleaving columns: xb = [even cols | odd cols], each
                # row shrinking to 128 unit-stride elems
                xb = pool_xb.tile([IMGS, MAX_K], bf16)
                src_eo = xt[:, 0:K].rearrange("p (n two) -> p two n", two=2)
                dst_eo = xb[:, 0:K].rearrange("p (two n) -> p two n", two=2)
                nc.scalar.mul(dst_eo, src_eo, 0.5)

                # vertical butterfly: row pairs (2i, 2i+1) within each of the
                # even/odd halves; every operand is a unit-stride 128-elem run
                # so the DVE runs in 2x packed mode
                uv = pool_uv.tile([IMGS, MAX_K], bf16)

                def rowpairs(buf, off):  # (p, i, [top|bot] 2, hw)
                    return buf[:, off : off + K // 2].rearrange(
                        "p (i two w) -> p i two w", two=2, w=hw
                    )

                xe = rowpairs(xb, 0)  # even cols
                xo = rowpairs(xb, K // 2)  # odd cols
                uvq = uv[:, 0:K].rearrange("p (k i w) -> p k i w", k=4, w=hw)
                u_e, u_o, v_e, v_o = (uvq[:, k] for k in range(4))
                te, be = xe[:, :, 0], xe[:, :, 1]
                to, bo = xo[:, :, 0], xo[:, :, 1]
                nc.vector.tensor_add(out=u_e, in0=te, in1=be)  # a+c
                nc.vector.tensor_add(out=u_o, in0=to, in1=bo)  # b+d
                nc.vector.tensor_sub(out=v_e, in0=be, in1=te)  # c-a
                nc.vector.tensor_sub(out=v_o, in0=bo, in1=to)  # d-b

                # horizontal butterfly: now plain contiguous q-elem arrays
                res = pool_res.tile([IMGS, MAX_K], bf16)
                ue_f, uo_f = uv[:, 0:q], uv[:, q : 2 * q]
                ve_f, vo_f = uv[:, 2 * q : 3 * q], uv[:, 3 * q : 4 * q]
                nc.vector.tensor_add(out=res[:, 0:q], in0=ue_f, in1=uo_f)  # cA
                nc.vector.tensor_add(out=res[:, q : 2 * q], in0=ve_f, in1=vo_f)  # cH
                nc.vector.tensor_sub(out=res[:, 2 * q : 3 * q], in0=uo_f, in1=ue_f)  # cV
                nc.vector.tensor_sub(out=res[:, 3 * q : 4 * q], in0=vo_f, in1=ve_f)  # cD

                # res = [cA|cH|cV|cD]; one strided store to all 4 subband regions
                o0 = (row0 // 2) * hw  # out offset within each subband
                dst = y_sub[:, :, o0 : o0 + q]  # (128, 4, q)
                src = res[:, 0:K].rearrange("p (k o) -> p k o", k=4)
                # stores ride the ACT HWDGE ring so loads stream uninterrupted
                # on the SP HWDGE ring
                nc.scalar.dma_start(out=dst, in_=src)
                row0 += rows
    # run Bacc's pass pipeline (regalloc, DCE, event-semaphore wait splitting)
    nc.compile()
    return nc


def _get_nc():
    if "nc" not in _CACHE:
        _CACHE["nc"] = build_nc()
    return _CACHE["nc"]


def _unshard(results):
    return np.concatenate(
        [
            np.asarray(r["y"]).astype(np.float32).reshape(B_PER, C * 4, H // 2, W // 2)
            for r in results
        ],
        axis=0,
    )


def kernel(x: np.ndarray) -> np.ndarray:
    from concourse.bass_utils import run_bass_kernel_spmd

    x = np.ascontiguousarray(np.asarray(x), dtype=np.float32)
    assert x.shape == (B, C, H, W), x.shape

    nc = _get_nc()
    in_maps = [
        {"x": x[c * B_PER : (c + 1) * B_PER].reshape(IMGS, IMG_PIX)}
        for c in range(N_CORES)
    ]
    results = run_bass_kernel_spmd(nc, in_maps, core_ids=list(range(N_CORES))).results
    return _unshard(results)
